# revision 2
# baseline (speedup 1.0000x reference)
"""Trainium2 Bass kernel v2 for nn_MultiHeadAttention (B=4, S=2048, D=1024, H=16).

Head-half sharding: 8 cores = (batch b in 0..3) x (head-half hh in 0..1).
Each core computes its 8 heads over the FULL 2048 queries:
  - Q/K/V projections with output dim 512 (its heads only)
  - attention for 8 heads (4 head-pair groups)
  - partial dense (contraction over its 512 ctx dims) -> [2048, 1024] fp32
Host sums the two partials per batch. This removes the K/V projection
redundancy of the (batch x query-half) sharding: tensor work drops from
786k to 655k cycles/core.

Schedule: ACT (exp) is the co-bottleneck (~256 instr x ~1.1us). Program
order interleaves one projection/dense "filler" matmul chunk after each
kt unit so the tensor engine stays busy while ACT paces attention, and
kq/V-proj/dense stream per-group:
  g0: fillers = V proj for g1..g3 + kq proj g1
  g1: fillers = kq g2;  g2: fillers = kq g3
  g3: fillers = dense for the previous qh chunk; dense(qh3) is the tail.
DMas are chunked per (i, tch) so scores(g0, qh0) can start ~7us in.

ctx PSUM tiles are copied to SBUF immediately (freeing the bank for the
next qh) and the softmax normalization (reciprocal of the ones-column
sums + gpsimd partition broadcast + multiply) runs off the critical path.
"""

import sys

for _p in ("/opt/trn_rl_repo", "/root/.axon_site/_ro/trn_rl_repo"):
    if _p not in sys.path:
        sys.path.insert(0, _p)

import os
import numpy as np
import ml_dtypes
_EXPCOPY = bool(int(os.environ.get("EXPCOPY", "0")))
_CTXSEQ = bool(int(os.environ.get("CTXSEQ", "0")))

import concourse.bacc as bacc
import concourse.bass as bass
import concourse.mybir as mybir
import concourse.tile as tile

B, S, D, H = 4, 2048, 1024, 16
DEPTH = 64
P = 128
NI = D // P             # 8 contraction chunks for projections
NGB = 4                 # head-pair groups per core (8 heads)
KT = S // P             # 16 key tiles
NQH = S // 512          # 4 query chunks of 512
F32 = mybir.dt.float32
BF16 = mybir.dt.bfloat16
BF16NP = ml_dtypes.bfloat16


def _build_bass(loop_k=None):
    nc = bacc.Bacc("TRN2", target_bir_lowering=False, debug=False)

    xqt = nc.dram_tensor("xqt", [D, S], BF16, kind="ExternalInput")
    xkt = nc.dram_tensor("xkt", [D, S], BF16, kind="ExternalInput")
    xvt = nc.dram_tensor("xvt", [D, S], BF16, kind="ExternalInput")
    wqt = nc.dram_tensor("wqt", [D, 512], BF16, kind="ExternalInput")
    wkt = nc.dram_tensor("wkt", [D, 512], BF16, kind="ExternalInput")
    wvt = nc.dram_tensor("wvt", [D, 512], BF16, kind="ExternalInput")
    dwt = nc.dram_tensor("dwt", [512, D], BF16, kind="ExternalInput")
    out = nc.dram_tensor("out", [S, D], F32, kind="ExternalOutput")

    xqt_ap, xkt_ap, xvt_ap = xqt.ap(), xkt.ap(), xvt.ap()
    wqt_ap, wkt_ap, wvt_ap, dwt_ap = wqt.ap(), wkt.ap(), wvt.ap(), dwt.ap()
    out_ap = out.ap()

    import contextlib

    with tile.TileContext(nc) as tc, nc.allow_low_precision(
            reason="bf16 operands are intentional"):
      with (tc.For_i(0, loop_k, 1,
                     hint_engines=(mybir.EngineType.PE,
                                   mybir.EngineType.Activation,
                                   mybir.EngineType.DVE))
            if loop_k else contextlib.nullcontext()):
        with (
            tc.tile_pool(name="resident", bufs=1) as resident,
            tc.tile_pool(name="xt", bufs=1) as xt_pool,
            tc.tile_pool(name="wt", bufs=1) as wt_pool,
            tc.tile_pool(name="kq", bufs=2) as kq_pool,
            tc.tile_pool(name="sb", bufs=1) as sb_pool,
            tc.tile_pool(name="scps", bufs=2, space="PSUM") as scps,
            tc.tile_pool(name="cps", bufs=1, space="PSUM") as cps,
            tc.tile_pool(name="mps", bufs=2, space="PSUM") as mps,
        ):
            # ---- resident tiles ----
            vh = resident.tile([P, KT, NGB, 2, DEPTH], BF16)
            ctxn = resident.tile([P, NGB, S], BF16)
            ones = resident.tile([P, 1], BF16)
            nc.gpsimd.memset(ones[:], 1.0)

            # ---- input tiles (loaded once, chunked DMA) ----
            xkt_t = xt_pool.tile([P, NI, S], BF16, tag="xkt")
            xqt_t = xt_pool.tile([P, NI, S], BF16, tag="xqt")
            xvt_t = xt_pool.tile([P, NI, S], BF16, tag="xvt")
            wkt_t = wt_pool.tile([P, NI, 512], BF16, tag="wkt")
            wqt_t = wt_pool.tile([P, NI, 512], BF16, tag="wqt")
            wvt_t = wt_pool.tile([P, NI, 512], BF16, tag="wvt")
            dwt_t = wt_pool.tile([P, NGB, D], BF16, tag="dwt")

            # DMA priority order: everything kq(g0, tch0) needs first.
            # Batched: few large strided transfers (each dma_start costs
            # ~650ns of serialized issue on SP).
            def xin(ap, cs):  # [1024, S] slice -> [128, NI, cs] pattern
                return ap[:, cs].rearrange("(i p) n -> p i n", p=P)

            nc.sync.dma_start(out=wkt_t[:, :, 0:P],
                              in_=xin(wkt_ap, slice(0, P)))
            nc.sync.dma_start(out=wqt_t[:, :, 0:P],
                              in_=xin(wqt_ap, slice(0, P)))
            nc.sync.dma_start(out=xkt_t[:, :, 0:1024],
                              in_=xin(xkt_ap, slice(0, 1024)))
            nc.sync.dma_start(out=xqt_t[:, :, 0:512],
                              in_=xin(xqt_ap, slice(0, 512)))
            nc.sync.dma_start(out=wkt_t[:, :, P:512],
                              in_=xin(wkt_ap, slice(P, 512)))
            nc.sync.dma_start(out=wqt_t[:, :, P:512],
                              in_=xin(wqt_ap, slice(P, 512)))
            nc.sync.dma_start(out=xkt_t[:, :, 1024:2048],
                              in_=xin(xkt_ap, slice(1024, 2048)))
            nc.sync.dma_start(
                out=wvt_t[:], in_=wvt_ap.rearrange("(i p) n -> p i n", p=P))
            nc.sync.dma_start(out=xvt_t[:, :, 0:1024],
                              in_=xin(xvt_ap, slice(0, 1024)))
            nc.sync.dma_start(out=xvt_t[:, :, 1024:2048],
                              in_=xin(xvt_ap, slice(1024, 2048)))
            nc.sync.dma_start(out=xqt_t[:, :, 512:1024],
                              in_=xin(xqt_ap, slice(512, 1024)))
            nc.sync.dma_start(out=xqt_t[:, :, 1024:2048],
                              in_=xin(xqt_ap, slice(1024, 2048)))
            nc.sync.dma_start(
                out=dwt_t[:], in_=dwt_ap.rearrange("(g p) n -> p g n", p=P))

            # HAM warm-up: keep the PE busy during the DMA lead-in so
            # the clock gate is at 8/8 when real matmuls start.
            # Results are discarded.
            warm_src = resident.tile([P, 512], BF16, name="warm_src")
            nc.gpsimd.memset(warm_src[:], 0.0)
            for w in range(24):
                wp = mps.tile([P, 512], F32, tag="mm", name="warm")
                nc.tensor.matmul(wp[:], warm_src[:, 0:P],
                                 warm_src[:], start=True, stop=True)

            # ---- emit helpers ----
            def emit_k_tch(khtg, g, tch):
                pj = mps.tile([P, 512], F32, tag="mm")
                for i in range(NI):
                    nc.tensor.matmul(
                        pj[:],
                        (wkt_t[:, i, g * P:(g + 1) * P]),
                        (xkt_t[:, i, tch * 512:(tch + 1) * 512]),
                        start=(i == 0), stop=(i == NI - 1))
                nc.vector.tensor_copy(
                    out=khtg[:, tch * 512:(tch + 1) * 512], in_=pj[:])

            def emit_q_tch(qhtg, g, tch):
                pj = mps.tile([P, 512], F32, tag="mm")
                for i in range(NI):
                    nc.tensor.matmul(
                        pj[:],
                        (wqt_t[:, i, g * P:(g + 1) * P]),
                        (xqt_t[:, i, tch * 512:(tch + 1) * 512]),
                        start=(i == 0), stop=(i == NI - 1))
                nc.vector.tensor_copy(
                    out=qhtg[:, tch * 512:(tch + 1) * 512], in_=pj[:])

            def emit_vp(j, c0, c1):
                # V proj for token tile j, weight cols c0:c1 (vh groups)
                pv = mps.tile([P, 512], F32, tag="mm")
                nw = c1 - c0
                for i in range(NI):
                    nc.tensor.matmul(
                        pv[:, 0:nw],
                        (xvt_t[:, i, j * P:(j + 1) * P]),
                        (wvt_t[:, i, c0:c1]),
                        start=(i == 0), stop=(i == NI - 1))
                # cols c0:c1 map to vh[:, j, g, h, :] blocks
                g0, g1 = c0 // P, c1 // P
                nc.vector.tensor_copy(
                    out=vh[:, j, g0:g1, :, :],
                    in_=pv[:, 0:nw])

            dnos = {}

            def emit_dense(st, ncp):
                dn = mps.tile([P, 512], F32, tag="mm")
                for g in range(NGB):
                    nc.tensor.matmul(
                        dn[:],
                        (ctxn[:, g, st * P:(st + 1) * P]),
                        (dwt_t[:, g, ncp * 512:(ncp + 1) * 512]),
                        start=(g == 0), stop=(g == NGB - 1))
                if ncp == 0:
                    dnos[st] = sb_pool.tile([P, D], F32, tag="dno",
                                            bufs=2, name="dno")
                dno = dnos[st]
                nc.vector.tensor_copy(
                    out=dno[:, ncp * 512:(ncp + 1) * 512], in_=dn[:])
                if ncp == 1:
                    nc.sync.dma_start(
                        out=out_ap[st * P:(st + 1) * P, :], in_=dno[:])

            # ---- build kq(g0) with minimal lead-in ----
            khtgs = {}
            qhtgs = {}

            def new_kq(g):
                khtgs[g] = kq_pool.tile([P, S], BF16, tag="khtg",
                                        name="khtg")
                qhtgs[g] = kq_pool.tile([P, S], BF16, tag="qhtg",
                                        name="qhtg")

            new_kq(0)
            emit_k_tch(khtgs[0], 0, 0)
            emit_q_tch(qhtgs[0], 0, 0)
            for tch in range(1, 4):
                emit_k_tch(khtgs[0], 0, tch)
            # V proj (full width) for the first key tile; rest stream
            # as fillers just ahead of each ctx(kt) in qh0
            emit_vp(0, 0, 512)

            # ---- filler schedule ----
            # fillers[(g, qh)] = list of thunks, one consumed per kt
            fillers = {}
            for g in range(NGB):
                for qh in range(NQH):
                    fillers[(g, qh)] = []

            def vp_thunk(j, c0, c1):
                return lambda: emit_vp(j, c0, c1)

            def k_thunk(g, tch):
                return lambda: emit_k_tch(khtgs[g], g, tch)

            def q_thunk(g, tch):
                return lambda: emit_q_tch(qhtgs[g], g, tch)

            # (0,qh0): full-width V proj j1..15 just ahead of each
            # ctx(kt), then qhtg tch1 for the next qh block.
            for j in range(1, KT):
                fillers[(0, 0)].append(vp_thunk(j, 0, 512))
            fillers[(0, 0)].append(q_thunk(0, 1))
            fillers[(0, 1)].append(q_thunk(0, 2))
            fillers[(0, 1)].append(k_thunk(1, 0))
            fillers[(0, 1)].append(k_thunk(1, 1))
            fillers[(0, 2)].append(q_thunk(0, 3))
            fillers[(0, 2)].append(k_thunk(1, 2))
            fillers[(0, 2)].append(k_thunk(1, 3))
            fillers[(0, 2)].append(q_thunk(1, 0))
            fillers[(0, 3)].append(q_thunk(1, 1))
            fillers[(0, 3)].append(q_thunk(1, 2))
            fillers[(0, 3)].append(q_thunk(1, 3))

            def kq_fillers(gsrc, gdst):
                # spread 8 proj chunks over the 4 qh blocks of group gsrc
                seq = [k_thunk(gdst, 0), k_thunk(gdst, 1),
                       k_thunk(gdst, 2), k_thunk(gdst, 3),
                       q_thunk(gdst, 0), q_thunk(gdst, 1),
                       q_thunk(gdst, 2), q_thunk(gdst, 3)]
                for idx, th in enumerate(seq):
                    fillers[(gsrc, idx // 2)].append(th)

            kq_fillers(1, 2)
            kq_fillers(2, 3)

            # g3: dense for qh-1 during qh (32 chunks of (st, ncp))
            def d_thunk(st, ncp):
                return lambda: emit_dense(st, ncp)
            for qh in range(1, NQH):
                prev = qh - 1
                for st in range(prev * 4, prev * 4 + 4):
                    for ncp in range(2):
                        fillers[(3, qh)].append(d_thunk(st, ncp))

            # ---- attention ----
            for g in range(NGB):
                if g + 1 < NGB:
                    new_kq(g + 1)
                khtg, qhtg = khtgs[g], qhtgs[g]
                for qh in range(NQH):
                    qs = slice(qh * 512, (qh + 1) * 512)
                    fl = list(fillers[(g, qh)])
                    fi = 0
                    # ctx2: h0 ctx in partitions 0:64 (col group 0),
                    # h1 ctx in partitions 64:128 (col group 1) -- the
                    # two ctx matmuls run CONCURRENTLY via PE col tiling
                    if _CTXSEQ:
                        ctxA_t = cps.tile([DEPTH, 512], F32, tag="cA",
                                          bufs=1, name="cA")
                        ctxB_t = cps.tile([DEPTH, 512], F32, tag="cB",
                                          bufs=1, name="cB")
                        ctx_parts = (ctxA_t, ctxB_t)
                    else:
                        ctx2 = cps.tile([P, 512], F32, tag="ctx2",
                                        bufs=2)
                        ctx_parts = (ctx2[0:DEPTH, :], ctx2[DEPTH:P, :])
                    # acc: running bf16 sum of at tiles (for softmax
                    # denominators), accumulated on DVE off-path
                    acc = sb_pool.tile([P, 1024], BF16, tag="acc",
                                       bufs=2, name="acc")
                    for kt in range(KT):
                        sc = scps.tile([P, 1024], F32, tag="sc")
                        nc.tensor.matmul(
                            sc[:, 0:512],
                            (khtg[0:DEPTH, kt * P:(kt + 1) * P]),
                            (qhtg[0:DEPTH, qs]),
                            start=True, stop=True)
                        nc.tensor.matmul(
                            sc[:, 512:1024],
                            (khtg[DEPTH:P, kt * P:(kt + 1) * P]),
                            (qhtg[DEPTH:P, qs]),
                            start=True, stop=True)
                        at = sb_pool.tile([P, 1024], BF16, tag="at",
                                          bufs=5)
                        if _EXPCOPY:
                            nc.scalar.copy(out=at[:], in_=sc[:])
                        else:
                            nc.scalar.activation(
                                at[:], sc[:],
                                mybir.ActivationFunctionType.Exp,
                                scale=0.125)
                        nc.tensor.matmul(
                            ctx_parts[0], (vh[:, kt, g, 0, :]),
                            (at[:, 0:512]),
                            start=(kt == 0), stop=(kt == KT - 1))
                        nc.tensor.matmul(
                            ctx_parts[1], (vh[:, kt, g, 1, :]),
                            (at[:, 512:1024]),
                            start=(kt == 0), stop=(kt == KT - 1))
                        if kt == 0:
                            nc.vector.tensor_copy(out=acc[:], in_=at[:])
                        else:
                            nc.vector.tensor_add(acc[:], acc[:], at[:])
                        # one filler chunk per kt keeps PE busy while
                        # ACT paces the loop
                        n_f = (len(fl) * (kt + 1)) // KT
                        while fi < n_f:
                            fl[fi]()
                            fi += 1
                    while fi < len(fl):
                        fl[fi]()
                        fi += 1

                    # softmax denominators: ones^T @ acc -> [1, 1024]
                    # in PSUM (2x 512), then broadcast + reciprocal +
                    # multiply straight out of ctx2 PSUM into ctxn
                    sm0 = mps.tile([1, 512], F32, tag="mm", name="sm0")
                    sm1 = mps.tile([1, 512], F32, tag="mm", name="sm1")
                    nc.tensor.matmul(sm0[:], ones[:], acc[:, 0:512],
                                     start=True, stop=True)
                    nc.tensor.matmul(sm1[:], ones[:], acc[:, 512:1024],
                                     start=True, stop=True)
                    # reciprocal PSUM->SBUF (gpsimd can't read PSUM),
                    # then broadcast to the ctx2 partition layout
                    rs = sb_pool.tile([1, 1024], F32, tag="rs",
                                      bufs=1, name="rs")
                    nc.vector.reciprocal(rs[:, 0:512], sm0[:])
                    nc.vector.reciprocal(rs[:, 512:1024], sm1[:])
                    rbcs0 = sb_pool.tile([DEPTH, 512], F32, tag="rbcs0",
                                         bufs=1, name="rbcs0")
                    rbcs1 = sb_pool.tile([DEPTH, 512], F32, tag="rbcs1",
                                         bufs=1, name="rbcs1")
                    nc.gpsimd.partition_broadcast(rbcs0[:], rs[:, 0:512])
                    nc.gpsimd.partition_broadcast(rbcs1[:],
                                                  rs[:, 512:1024])
                    nc.vector.tensor_mul(
                        ctxn[0:DEPTH, g, qs], ctx_parts[0], rbcs0[:])
                    nc.vector.tensor_mul(
                        ctxn[DEPTH:P, g, qs], ctx_parts[1], rbcs1[:])

            # dense tail: qh3
            for st in range(12, 16):
                for ncp in range(2):
                    emit_dense(st, ncp)

    nc.finalize()
    return nc


_CACHE = {}


def _get_runner(loop_k=None):
    """Build the Bass module once and return a cached jitted SPMD runner."""
    key = ("runner", loop_k)
    if key in _CACHE:
        return _CACHE[key]

    import jax
    from jax.sharding import Mesh, PartitionSpec
    from jax.experimental.shard_map import shard_map
    from concourse import bass2jax

    nc = _build_bass(loop_k=loop_k)
    bass2jax.install_neuronx_cc_hook()

    partition_name = (nc.partition_id_tensor.name
                      if nc.partition_id_tensor else None)
    in_names, out_names, out_avals, zero_shapes = [], [], [], []
    for alloc in nc.m.functions[0].allocations:
        if not isinstance(alloc, mybir.MemoryLocationSet):
            continue
        name = alloc.memorylocations[0].name
        if alloc.kind == "ExternalInput":
            if name != partition_name:
                in_names.append(name)
        elif alloc.kind == "ExternalOutput":
            shape = tuple(alloc.tensor_shape)
            dtype = mybir.dt.np(alloc.dtype)
            out_avals.append(jax.core.ShapedArray(shape, dtype))
            out_names.append(name)
            zero_shapes.append((shape, dtype))
    n_params = len(in_names)
    n_outs = len(out_avals)
    all_in_names = list(in_names) + list(out_names)
    if partition_name is not None:
        all_in_names.append(partition_name)

    def _body(*args):
        operands = list(args)
        if partition_name is not None:
            operands.append(bass2jax.partition_id_tensor())
        outs = bass2jax._bass_exec_p.bind(
            *operands,
            out_avals=tuple(out_avals),
            in_names=tuple(all_in_names),
            out_names=tuple(out_names),
            lowering_input_output_aliases=(),
            sim_require_finite=True,
            sim_require_nnan=True,
            nc=nc,
        )
        return tuple(outs)

    n_cores = 8
    devices = jax.devices()[:n_cores]
    mesh = Mesh(np.asarray(devices), ("core",))
    in_specs = (PartitionSpec("core"),) * (n_params + n_outs)
    out_specs = (PartitionSpec("core"),) * n_outs
    donate = tuple(range(n_params, n_params + n_outs))
    sharded = jax.jit(
        shard_map(_body, mesh=mesh, in_specs=in_specs, out_specs=out_specs,
                  check_rep=False),
        donate_argnums=donate, keep_unused=True)

    def runner(in_maps):
        per_core = [[np.asarray(m[name]) for name in in_names]
                    for m in in_maps]
        concat_in = [np.concatenate([per_core[c][i] for c in range(n_cores)],
                                    axis=0) for i in range(n_params)]
        concat_zeros = [np.zeros((n_cores * s[0], *s[1:]), d)
                        for s, d in zero_shapes]
        out_arrs = sharded(*concat_in, *concat_zeros)
        return [
            {name: np.asarray(out_arrs[i]).reshape(
                n_cores, *out_avals[i].shape)[c]
             for i, name in enumerate(out_names)}
            for c in range(n_cores)
        ]

    runner.sharded = sharded
    runner.in_names = in_names
    runner.out_names = out_names
    runner.zero_shapes = zero_shapes
    runner.n_cores = n_cores
    _CACHE[key] = runner
    return runner


def _shard_inputs(inputs):
    q = np.asarray(inputs["q"], np.float32)
    k = np.asarray(inputs["k"], np.float32)
    v = np.asarray(inputs["v"], np.float32)

    def t16(a):  # [r, c] -> bf16 contiguous transpose [c, r]
        return np.ascontiguousarray(np.asarray(a, np.float32).T).astype(BF16NP)

    wqT = t16(inputs["wq_w"])     # [1024 din, 1024 douts]
    wkT = t16(inputs["wk_w"])
    wvT = t16(inputs["wv_w"])
    dwT = t16(inputs["dense_w"])  # [1024 din(ctx dims), 1024 douts]

    in_maps = []
    for c in range(8):
        b, hh = c // 2, c % 2
        cs = slice(hh * 512, (hh + 1) * 512)
        m = {
            "xqt": t16(q[b]),
            "xkt": t16(k[b]),
            "xvt": t16(v[b]),
            "wqt": np.ascontiguousarray(wqT[:, cs]),
            "wkt": np.ascontiguousarray(wkT[:, cs]),
            "wvt": np.ascontiguousarray(wvT[:, cs]),
            "dwt": np.ascontiguousarray(dwT[cs, :]),
        }
        in_maps.append(m)
    return in_maps


def kernel(**inputs):
    runner = _get_runner()
    in_maps = _shard_inputs(inputs)
    results = runner(in_maps)
    output = np.empty((B, S, D), np.float32)
    for b in range(B):
        output[b] = results[2 * b]["out"] + results[2 * b + 1]["out"]
    return output
